# revision 18
# baseline (speedup 1.0000x reference)
"""Bass/Trainium2 kernel for nn_DirectedMessagePassingLayer_65807488909810.

Reference computation:
    agg_in  = segment_sum(vals_in[:,None]  * x[cols_in],  rows_in,  n)
    agg_out = segment_sum(vals_out[:,None] * x[cols_out], rows_out, n)
    h = x @ W_self.T + b_self + agg_in @ W_in.T + agg_out @ W_out.T
    out = relu(layernorm(h) * gamma + beta)        # gamma=1, beta=0 handled

Distribution (8 NeuronCores, SPMD — one compiled program, per-core data):
  nodes (rows of x / output) are sharded 6250/core; edges are partitioned by
  destination row so the segment-sum is core-local; x is replicated as a
  row-major bf16 gather table (lo/hi split for int16 indices); weights/LN
  params replicated.

Per-core algorithm (v2 — shared-chunk layout):
  * Edges bucket into cells (block b, set s, dest-window w of 64). Cell sizes
    are equalized across cores: the lo-stream size is PINNED per cell by
    rebalancing "flex" edges (cols in [HI_BASE, XLO_ROWS) fit either table);
    the hi stream pads to the max over cores. No per-cell 128-alignment —
    cells share 128-slot chunks, cutting slot padding from ~20% to ~5%.
  * Slots are gathered from the bf16 x tables with InstDMAGatherAnt in
    batches of 128*SB rows.
  * Per (chunk, overlapping cell): a scaled one-hot S[p, j] =
    val[p] * (iota_par[j] == rl'[p]) is built in ONE fused DVE op
    (tensor_scalar is_equal+mult, 4x perf mode). Cells are parity-tagged
    (rl' = rl + 64*(cell%4), iota windows [64k, 64k+64)) so slots of other
    cells in the same chunk never match — masked to zero.
  * The tensor engine scatters each (chunk, cell) into the cell's PSUM
    region: PSUM_agg[s][feat, w*64:] += G_chunk.T @ S, with start/stop
    bracketing the (b,s,w) accumulation group across both streams.
  * Per block: hT = WselfT.T @ xT_blk + WinT.T @ aggT_in + WoutT.T @ aggT_out
    accumulated in PSUM, bias added, PE-transposed, layer-normed (free-dim
    stats) + relu'd, stored.
"""

import numpy as np
import ml_dtypes

import concourse.bass as bass
import concourse.bacc as bacc
import concourse.mybir as mybir
import concourse.tile as tile
from concourse.bass_utils import run_bass_kernel_spmd
from concourse.masks import make_identity

# ---------------- problem constants (hardcoded per contract) ----------------
N_NODES = 50000
D = 128
LN_EPS = 1e-5
N_CORES = 8
ROWS_PER_CORE = 6250
BLOCKS = 49                   # ceil(6250/128)
PAD_ROWS = BLOCKS * 128       # 6272
WIN = 64                      # dest window width
CHUNK = 128
SB = 32                       # gather batch stripes (4096-row gathers)
XLO_ROWS = 32768              # lo table = x[0:XLO_ROWS]
HI_BASE = 17232               # hi table = x[HI_BASE:] (32768 rows)
MINC = 43                     # min cell size: <=4 cells touch any chunk
NSETS = 2
NCELLS = BLOCKS * NSETS * 2   # cells per stream; order b-major, then (s, w)

F32 = mybir.dt.float32
BF16 = mybir.dt.bfloat16
I16 = mybir.dt.int16


def _split_multi_waits(nc):
    """This walrus build encodes at most one sync-wait per instruction;
    split N-wait instructions into N-1 preceding single-wait NoOps
    (engine-serial execution preserves the semantics)."""
    k = 0
    for f in nc.m.functions:
        for bb in f.blocks:
            new = []
            for inst in bb.instructions:
                si = inst.sync_info
                if si is not None and si.on_wait is not None and len(si.on_wait) > 1:
                    waits = list(si.on_wait)
                    for w in waits[:-1]:
                        k += 1
                        new.append(mybir.InstNoOp(
                            name=f"waitsplit-{k}", engine=inst.engine,
                            ins=[], outs=[],
                            sync_info=mybir.SyncInfo(on_wait=[w], on_update=[])))
                    si.on_wait = waits[-1:]
                new.append(inst)
            bb.instructions = new
    return k


def _cell_of(b, s, w):
    return (b * NSETS + s) * 2 + w


def _wrap_slots(a):
    """[stripes*128] -> [128, stripes] with slot g at [g%128, g//128]."""
    return np.ascontiguousarray(a.reshape(-1, 128).T)


def _wrap_idx16(a, batches):
    """Per-batch 16-wrap of int16 indices for dma_gather.

    `a`: [total_slots]; `batches`: list of stripe counts per batch.
    Batch bi with nstr stripes occupies nstr*8 columns: in-batch index j ->
    [j%16 (replicated x8 in partitions), col0 + j//16].
    """
    blocks = []
    pos = 0
    for nstr in batches:
        n = nstr * 128
        A = a[pos:pos + n].reshape(n // 16, 16)
        B = np.tile(A.T, (8, 1))                      # [128, n//16]
        blocks.append(B)
        pos += n
    assert pos == len(a)
    return np.ascontiguousarray(np.concatenate(blocks, axis=1))


def _build_layout(edge_sets):
    """v2 shared-chunk layout. Returns per-core streams + shared program."""
    m_cnt = np.zeros((N_CORES, NCELLS), np.int64)   # must_lo
    f_cnt = np.zeros((N_CORES, NCELLS), np.int64)   # flex
    t_cnt = np.zeros((N_CORES, NCELLS), np.int64)   # total
    edata = []
    for s, (rows, cols, vals) in enumerate(edge_sets):
        core = rows // ROWS_PER_CORE
        rloc = rows - core * ROWS_PER_CORE
        b = rloc >> 7
        w = (rloc >> 6) & 1
        rl = rloc & 63
        cell = _cell_of(b, s, w)
        cc = core * NCELLS + cell
        must_lo = cols < HI_BASE
        flex = (cols >= HI_BASE) & (cols < XLO_ROWS)
        m_cnt += np.bincount(cc[must_lo], minlength=N_CORES * NCELLS).reshape(
            N_CORES, NCELLS)
        f_cnt += np.bincount(cc[flex], minlength=N_CORES * NCELLS).reshape(
            N_CORES, NCELLS)
        t_cnt += np.bincount(cc, minlength=N_CORES * NCELLS).reshape(
            N_CORES, NCELLS)
        edata.append((core, cell, rl, cols, vals, must_lo, flex))

    # per-cell lo pin: minimize pin + max_core(total - min(pin, avail))
    avail = m_cnt + f_cnt
    lo_pin = np.zeros(NCELLS, np.int64)
    for c in range(NCELLS):
        lo0 = int(m_cnt[:, c].max())
        hi0 = int(avail[:, c].max())
        pins = np.arange(lo0, hi0 + 1)
        if len(pins) == 0:
            lo_pin[c] = max(lo0, MINC)
            continue
        rest = t_cnt[:, c][None, :] - np.minimum(pins[:, None], avail[:, c][None, :])
        costs = pins + rest.max(axis=1)
        lo_pin[c] = max(int(pins[np.argmin(costs)]), MINC)
    lo_real = np.minimum(lo_pin[None, :], avail)
    hi_cnt = t_cnt - lo_real
    hi_size = np.maximum(hi_cnt.max(axis=0), MINC)

    sizes = [lo_pin, hi_size]
    starts = [np.concatenate([[0], np.cumsum(sz)[:-1]]) for sz in sizes]
    lens = [int(sz.sum()) for sz in sizes]
    plens = [-(-L // CHUNK) * CHUNK for L in lens]

    idx = [np.zeros((N_CORES, P), np.int64) for P in plens]
    rlp = [np.full((N_CORES, P), 999.0, np.float32) for P in plens]
    val = [np.zeros((N_CORES, P), np.float32) for P in plens]

    for s in range(NSETS):
        core, cell, rl, cols, vals, must_lo, flex = edata[s]
        order = np.lexsort((np.where(must_lo, 0, np.where(flex, 1, 2)),
                            cell, core))
        co, ce = core[order], cell[order]
        rlo, clo, vlo = rl[order], cols[order], vals[order]
        grp = co * NCELLS + ce
        gstart = np.zeros(N_CORES * NCELLS + 1, np.int64)
        np.add.at(gstart, grp + 1, 1)
        gstart = np.cumsum(gstart)
        rank = np.arange(len(order)) - gstart[grp]
        is_lo = rank < lo_real[co, ce]
        assert not np.any(is_lo & ~(must_lo[order] | flex[order]))
        for h in range(2):
            sel = is_lo if h == 0 else ~is_lo
            coh, ceh = co[sel], ce[sel]
            rk = rank[sel] if h == 0 else rank[sel] - lo_real[coh, ceh]
            pos = starts[h][ceh] + rk
            assert np.all(rk >= 0) and np.all(rk < sizes[h][ceh])
            idx[h][coh, pos] = clo[sel] - (HI_BASE if h else 0)
            rlp[h][coh, pos] = rlo[sel] + 64.0 * (ceh % 4)
            val[h][coh, pos] = vlo[sel]

    # shared program: per stream chunk -> [cell, ...]
    chunk_cells = []
    for h in range(2):
        nchunk = plens[h] // CHUNK
        ops = [[] for _ in range(nchunk)]
        for c in range(NCELLS):
            st, en = int(starts[h][c]), int(starts[h][c] + sizes[h][c])
            for k in range(st // CHUNK, (en - 1) // CHUNK + 1):
                ops[k].append(c)
        chunk_cells.append(ops)
    for h in range(2):
        for k, ops in enumerate(chunk_cells[h]):
            ps = [c % 4 for c in ops]
            assert len(set(ps)) == len(ps), ("parity clash", h, k, ops)

    consume = []
    for b in range(BLOCKS):
        ends = []
        for h in range(2):
            last_cell = _cell_of(b, NSETS - 1, 1)
            en = int(starts[h][last_cell] + sizes[h][last_cell])
            ends.append(-(-en // CHUNK))
        consume.append(tuple(ends))
    consume[-1] = (plens[0] // CHUNK, plens[1] // CHUNK)

    # region-major program: each (b,s,w) region's ops run consecutively
    # (lo-cell chunks then hi-cell chunks) so PSUM accumulation groups in one
    # bank never interleave — start=True zeroes a whole 2KB zero region, and
    # the interpreter/HW forbids a second start while a group is pending.
    # Straddle chunks are simply revisited at each overlapping cell's turn.
    prog = []
    for b in range(BLOCKS):
        regions = []
        for s in range(NSETS):
            for w in range(2):
                c = _cell_of(b, s, w)
                visits = []
                for h in range(2):
                    st = int(starts[h][c])
                    en = st + int(sizes[h][c])
                    for k in range(st // CHUNK, (en - 1) // CHUNK + 1):
                        visits.append((h, k))
                ops = [(h, k, i == 0, i == len(visits) - 1)
                       for i, (h, k) in enumerate(visits)]
                regions.append((s, w, c, ops))
        prog.append(regions)

    # gather batches: stripe counts per batch (tail batch may be short)
    batches = []
    for h in range(2):
        stripes = plens[h] // CHUNK
        bl = []
        while stripes > 0:
            take = min(SB, stripes)
            if stripes <= 2 * SB:
                take = min(12, stripes)
            bl.append(take)
            stripes -= take
        batches.append(bl)

    # wrapped per-core arrays
    out = {"prog": prog, "batches": batches, "plens": plens}
    gdt = np.dtype(ml_dtypes.bfloat16)
    for h in range(2):
        out[f"idx{h}"] = np.stack(
            [_wrap_idx16(idx[h][ci].astype(np.int16), batches[h])
             for ci in range(N_CORES)])
        # rl/val interleaved: [128, 2*stripes], col 2g = rl', 2g+1 = val
        rl_w = np.stack([_wrap_slots(rlp[h][ci]) for ci in range(N_CORES)])
        va_w = np.stack([_wrap_slots(val[h][ci]) for ci in range(N_CORES)])
        rv = np.empty((N_CORES, 128, 2 * rl_w.shape[2]), np.float32)
        rv[:, :, 0::2] = rl_w
        rv[:, :, 1::2] = va_w
        out[f"rlv{h}"] = rv
    return out


def _trace_kernel(nc, lay, gamma_trivial, beta_trivial):
    prog = lay["prog"]
    batches = lay["batches"]
    plens = lay["plens"]
    stripes = [p // CHUNK for p in plens]
    icolumns = [p // 16 for p in plens]          # int16 idx columns

    xlo = nc.declare_dram_parameter("xlo", [XLO_ROWS, D], BF16, isOutput=False)
    xhi = nc.declare_dram_parameter("xhi", [N_NODES - HI_BASE, D], BF16,
                                    isOutput=False)
    xT = nc.declare_dram_parameter("xT", [D, PAD_ROWS], BF16, isOutput=False)
    WselfT = nc.declare_dram_parameter("WselfT", [D, D], BF16, isOutput=False)
    WinT = nc.declare_dram_parameter("WinT", [D, D], BF16, isOutput=False)
    WoutT = nc.declare_dram_parameter("WoutT", [D, D], BF16, isOutput=False)
    bself = nc.declare_dram_parameter("bself", [D, 1], F32, isOutput=False)
    iota4_d = nc.declare_dram_parameter("iota4", [128, 4 * WIN], BF16,
                                        isOutput=False)
    idx_d, rlv_d = [], []
    for h in range(2):
        idx_d.append(nc.declare_dram_parameter(
            f"idx{h}", [128, icolumns[h]], I16, isOutput=False))
        rlv_d.append(nc.declare_dram_parameter(
            f"rlv{h}", [128, 2 * stripes[h]], F32, isOutput=False))
    if not gamma_trivial:
        gamma_d = nc.declare_dram_parameter("gamma_rep", [128, D], F32,
                                            isOutput=False)
    if not beta_trivial:
        beta_d = nc.declare_dram_parameter("beta_rep", [128, D], F32,
                                           isOutput=False)
    out_d = nc.declare_dram_parameter("out", [PAD_ROWS, D], BF16, isOutput=True)

    xtab = [xlo, xhi]

    with tile.TileContext(nc) as tc:
        with (
            tc.tile_pool(name="const", bufs=1) as constp,
            tc.tile_pool(name="g0", bufs=3) as g0pool,
            tc.tile_pool(name="g1", bufs=3) as g1pool,
            tc.tile_pool(name="meta", bufs=4) as mpool,
            tc.tile_pool(name="stp", bufs=6) as stpool,
            tc.tile_pool(name="sbuf", bufs=3) as spool,
            tc.tile_pool(name="aggp", bufs=4) as aggpool,
            tc.tile_pool(name="outp", bufs=4) as opool,
            tc.tile_pool(name="psumA", bufs=3, space="PSUM") as psA,
            tc.tile_pool(name="psumH", bufs=2, space="PSUM") as psH,
        ):
            gpool = [g0pool, g1pool]
            # ---- constants (iota first; bulk consts issue after the first
            # gather batches so SP's serial DMA issue doesn't delay them) ----
            WselfT_s = constp.tile([D, D], BF16, tag="wself")
            WinT_s = constp.tile([D, D], BF16, tag="win")
            WoutT_s = constp.tile([D, D], BF16, tag="wout")
            bself_s = constp.tile([D, 1], F32, tag="bself")
            ident = constp.tile([128, 128], F32, tag="ident")
            xT_s = constp.tile([D, PAD_ROWS], BF16, tag="xt")
            iota4_s = constp.tile([128, 4 * WIN], BF16, tag="iota4")
            nc.sync.dma_start(out=iota4_s[:], in_=iota4_d[:])

            # ---- per-stream gather batches (random access, monotone-ish) ----
            made = [{}, {}]
            bstart = [np.concatenate([[0], np.cumsum(bl)]).astype(int)
                      for bl in batches]

            def make_batch(h, bi):
                nstr = batches[h][bi]
                gt = gpool[h].tile([128, SB, D], BF16, tag="g")
                it = mpool.tile([128, SB * 8], I16, tag=f"idx{h}")
                rv = mpool.tile([128, SB * 2], F32, tag=f"rlv{h}")
                c0 = int(bstart[h][bi]) * 8
                s0 = int(bstart[h][bi])
                nc.sync.dma_start(out=it[:, :nstr * 8],
                                  in_=idx_d[h][:, c0:c0 + nstr * 8])
                nc.sync.dma_start(out=rv[:, :nstr * 2],
                                  in_=rlv_d[h][:, s0 * 2:(s0 + nstr) * 2])
                nc.gpsimd.dma_gather(
                    out_ap=gt[:, :nstr, :], in_ap=xtab[h][:],
                    idxs_ap=it[:, :nstr * 8],
                    num_idxs=nstr * 128, num_idxs_reg=nstr * 128, elem_size=D,
                    single_packet=False)
                return gt, rv

            def chunk_tiles(h, k):
                bi = int(np.searchsorted(bstart[h], k, "right")) - 1
                if bi not in made[h]:
                    made[h][bi] = make_batch(h, bi)
                gt, rv = made[h][bi]
                off = k - int(bstart[h][bi])
                return gt[:, off, :], rv, off

            # prefetch the first batch of both streams, then bulk consts
            made[0][0] = make_batch(0, 0)
            made[1][0] = make_batch(1, 0)
            nc.sync.dma_start(out=WselfT_s[:], in_=WselfT[:])
            nc.sync.dma_start(out=WinT_s[:], in_=WinT[:])
            nc.sync.dma_start(out=WoutT_s[:], in_=WoutT[:])
            nc.sync.dma_start(out=bself_s[:], in_=bself[:])
            nc.sync.dma_start(out=xT_s[:], in_=xT[:])
            make_identity(nc, ident[:])
            if not gamma_trivial:
                gamma_s = constp.tile([128, D], F32, tag="gamma")
                nc.sync.dma_start(out=gamma_s[:], in_=gamma_d[:])
            if not beta_trivial:
                beta_s = constp.tile([128, D], F32, tag="beta")
                nc.sync.dma_start(out=beta_s[:], in_=beta_d[:])

            pa_tiles = {}

            def get_pa(b, s):
                key = (b, s)
                if key not in pa_tiles:
                    pa_tiles[key] = psA.tile([128, 128], F32, tag="pa",
                                             space="PSUM",
                                             name=f"pa_b{b}_s{s}")
                return pa_tiles[key]

            for b in range(BLOCKS):
                for (s, w, c, ops) in prog[b]:
                    pa = get_pa(b, s)
                    par = c % 4
                    for (h, k, first, last) in ops:
                        g_ap, rv, off = chunk_tiles(h, k)
                        st_t = stpool.tile([128, WIN], BF16, tag="st")
                        nc.vector.tensor_scalar(
                            out=st_t[:],
                            in0=iota4_s[:, par * WIN:(par + 1) * WIN],
                            scalar1=rv[:, 2 * off:2 * off + 1],
                            scalar2=rv[:, 2 * off + 1:2 * off + 2],
                            op0=mybir.AluOpType.is_equal,
                            op1=mybir.AluOpType.mult)
                        nc.tensor.matmul(
                            out=pa[:, w * WIN:(w + 1) * WIN],
                            lhsT=g_ap, rhs=st_t[:],
                            start=first, stop=last)

                # ---- block reduction ----
                aggs = []
                for s in range(NSETS):
                    pa = pa_tiles.pop((b, s))
                    agg_t = aggpool.tile([128, 128], BF16, tag="agg",
                                         name=f"agg_b{b}_s{s}")
                    nc.scalar.copy(out=agg_t[:], in_=pa[:])
                    aggs.append(agg_t)

                ph = psH.tile([128, 128], F32, tag="ph", space="PSUM")
                nc.tensor.matmul(out=ph[:], lhsT=WselfT_s[:],
                                 rhs=xT_s[:, b * 128:(b + 1) * 128],
                                 start=True, stop=False)
                nc.tensor.matmul(out=ph[:], lhsT=WinT_s[:], rhs=aggs[0][:],
                                 start=False, stop=False)
                nc.tensor.matmul(out=ph[:], lhsT=WoutT_s[:], rhs=aggs[1][:],
                                 start=False, stop=True)
                hT = spool.tile([128, 128], F32, tag="ht")
                nc.vector.tensor_scalar(out=hT[:], in0=ph[:],
                                        scalar1=bself_s[:, :1], scalar2=None,
                                        op0=mybir.AluOpType.add)
                pt = psH.tile([128, 128], F32, tag="pt", space="PSUM")
                nc.tensor.transpose(out=pt[:], in_=hT[:], identity=ident[:])

                # layernorm over free dim + relu
                ssum = spool.tile([128, 1], F32, tag="ssum")
                nc.vector.reduce_sum(out=ssum[:], in_=pt[:],
                                     axis=mybir.AxisListType.X)
                sq = spool.tile([128, 128], F32, tag="sq")
                sqsum = spool.tile([128, 1], F32, tag="sqsum")
                nc.scalar.activation(out=sq[:], in_=pt[:],
                                     func=mybir.ActivationFunctionType.Square,
                                     accum_out=sqsum[:])
                mu = spool.tile([128, 1], F32, tag="mu")
                nc.vector.tensor_scalar_mul(out=mu[:], in0=ssum[:],
                                            scalar1=1.0 / D)
                musq = spool.tile([128, 1], F32, tag="musq")
                nc.vector.tensor_tensor(out=musq[:], in0=mu[:], in1=mu[:],
                                        op=mybir.AluOpType.mult)
                var = spool.tile([128, 1], F32, tag="var")
                nc.vector.tensor_scalar(out=var[:], in0=sqsum[:],
                                        scalar1=1.0 / D, scalar2=LN_EPS,
                                        op0=mybir.AluOpType.mult,
                                        op1=mybir.AluOpType.add)
                nc.vector.tensor_tensor(out=var[:], in0=var[:], in1=musq[:],
                                        op=mybir.AluOpType.subtract)
                std = spool.tile([128, 1], F32, tag="std")
                nc.scalar.activation(out=std[:], in_=var[:],
                                     func=mybir.ActivationFunctionType.Sqrt)
                rstd = spool.tile([128, 1], F32, tag="rstd")
                nc.vector.reciprocal(out=rstd[:], in_=std[:])
                nrm = opool.tile([128, 128], F32, tag="nrm")
                nc.vector.tensor_scalar(out=nrm[:], in0=pt[:],
                                        scalar1=mu[:, :1], scalar2=rstd[:, :1],
                                        op0=mybir.AluOpType.subtract,
                                        op1=mybir.AluOpType.mult)
                if not gamma_trivial:
                    nc.vector.tensor_tensor(out=nrm[:], in0=nrm[:],
                                            in1=gamma_s[:],
                                            op=mybir.AluOpType.mult)
                if not beta_trivial:
                    nc.vector.tensor_tensor(out=nrm[:], in0=nrm[:],
                                            in1=beta_s[:],
                                            op=mybir.AluOpType.add)
                ot = opool.tile([128, 128], BF16, tag="o")
                nc.scalar.activation(out=ot[:], in_=nrm[:],
                                     func=mybir.ActivationFunctionType.Relu)
                nc.sync.dma_start(out=out_d[b * 128:(b + 1) * 128, :],
                                  in_=ot[:])


def build(x, adj_in_rows, adj_in_cols, adj_in_vals,
          adj_out_rows, adj_out_cols, adj_out_vals,
          W_self, b_self, W_in, W_out, ln_gamma, ln_beta):
    """Trace + compile; returns (nc, in_maps)."""
    x = np.asarray(x, dtype=np.float32)
    sets = [
        (np.asarray(adj_in_rows, np.int64), np.asarray(adj_in_cols, np.int64),
         np.asarray(adj_in_vals, np.float32)),
        (np.asarray(adj_out_rows, np.int64), np.asarray(adj_out_cols, np.int64),
         np.asarray(adj_out_vals, np.float32)),
    ]
    W_self = np.asarray(W_self, np.float32)
    W_in = np.asarray(W_in, np.float32)
    W_out = np.asarray(W_out, np.float32)
    b_self = np.asarray(b_self, np.float32)
    ln_gamma = np.asarray(ln_gamma, np.float32)
    ln_beta = np.asarray(ln_beta, np.float32)

    lay = _build_layout(sets)
    gamma_trivial = bool(np.all(ln_gamma == 1.0))
    beta_trivial = bool(np.all(ln_beta == 0.0))

    nc = bacc.Bacc("TRN2", target_bir_lowering=False, debug=False,
                   num_devices=N_CORES, dynamic_dma_scratch_size=81920)
    _trace_kernel(nc, lay, gamma_trivial, beta_trivial)
    nc.compile()

    gdt = np.dtype(ml_dtypes.bfloat16)
    xlo = np.ascontiguousarray(x[:XLO_ROWS]).astype(gdt)
    xhi = np.ascontiguousarray(x[HI_BASE:]).astype(gdt)
    iota4 = np.tile(np.arange(4 * WIN, dtype=np.float32)[None, :],
                    (128, 1)).astype(gdt)
    in_maps = []
    for ci in range(N_CORES):
        r0 = ci * ROWS_PER_CORE
        xT_c = np.zeros((D, PAD_ROWS), dtype=gdt)
        xT_c[:, :ROWS_PER_CORE] = x[r0:r0 + ROWS_PER_CORE].T.astype(gdt)
        m = {
            "xlo": xlo, "xhi": xhi, "xT": xT_c, "iota4": iota4,
            "WselfT": np.ascontiguousarray(W_self.T).astype(gdt),
            "WinT": np.ascontiguousarray(W_in.T).astype(gdt),
            "WoutT": np.ascontiguousarray(W_out.T).astype(gdt),
            "bself": np.ascontiguousarray(b_self[:, None]),
        }
        for h in range(2):
            m[f"idx{h}"] = lay[f"idx{h}"][ci]
            m[f"rlv{h}"] = lay[f"rlv{h}"][ci]
        if not gamma_trivial:
            m["gamma_rep"] = np.tile(ln_gamma[None, :], (128, 1))
        if not beta_trivial:
            m["beta_rep"] = np.tile(ln_beta[None, :], (128, 1))
        in_maps.append(m)
    return nc, in_maps


def kernel(**inputs):
    nc, in_maps = build(**inputs)
    _split_multi_waits(nc)
    res = run_bass_kernel_spmd(nc, in_maps, core_ids=list(range(N_CORES)))
    out = np.concatenate(
        [res.results[ci]["out"][:ROWS_PER_CORE] for ci in range(N_CORES)],
        axis=0)
    return out.astype(np.float32)


def make_timed_runner(nc, in_maps, n_cores):
    """Jitted 8-core SPMD executable with repeat-callable timing (mirrors
    concourse.bass2jax.run_bass_via_pjrt's multi-core path)."""
    import time
    import jax
    from jax.experimental.shard_map import shard_map
    from jax.sharding import Mesh, PartitionSpec, NamedSharding
    from concourse.bass2jax import _bass_exec_p, install_neuronx_cc_hook, \
        partition_id_tensor

    install_neuronx_cc_hook()
    partition_name = nc.partition_id_tensor.name if nc.partition_id_tensor else None
    in_names, out_names, out_avals, zero_outs = [], [], [], []
    for alloc in nc.m.functions[0].allocations:
        if not isinstance(alloc, mybir.MemoryLocationSet):
            continue
        name = alloc.memorylocations[0].name
        if alloc.kind == "ExternalInput":
            if name != partition_name:
                in_names.append(name)
        elif alloc.kind == "ExternalOutput":
            shape = tuple(alloc.tensor_shape)
            dtype = mybir.dt.np(alloc.dtype)
            out_names.append(name)
            out_avals.append(jax.core.ShapedArray(shape, dtype))
            zero_outs.append(np.zeros(shape, dtype))
    n_params, n_outs = len(in_names), len(out_avals)
    all_in_names = list(in_names) + list(out_names)
    if partition_name is not None:
        all_in_names.append(partition_name)

    def _body(*args):
        operands = list(args)
        if partition_name is not None:
            operands.append(partition_id_tensor())
        return tuple(_bass_exec_p.bind(
            *operands, out_avals=tuple(out_avals), in_names=tuple(all_in_names),
            out_names=tuple(out_names), lowering_input_output_aliases=(),
            sim_require_finite=True, sim_require_nnan=True, nc=nc))

    devices = jax.devices()[:n_cores]
    mesh = Mesh(np.asarray(devices), ("core",))
    in_specs = (PartitionSpec("core"),) * (n_params + n_outs)
    out_specs = (PartitionSpec("core"),) * n_outs
    sharded = jax.jit(
        shard_map(_body, mesh=mesh, in_specs=in_specs, out_specs=out_specs,
                  check_rep=False),
        donate_argnums=tuple(range(n_params, n_params + n_outs)),
        keep_unused=True)
    shard0 = NamedSharding(mesh, PartitionSpec("core"))
    dev_in = [jax.device_put(
        np.concatenate([np.asarray(in_maps[c][nm]) for c in range(n_cores)],
                       axis=0),
        shard0) for nm in in_names]
    concat_zeros = [np.zeros((n_cores * z.shape[0], *z.shape[1:]), z.dtype)
                    for z in zero_outs]

    def run():
        dev_zeros = [jax.device_put(a, shard0) for a in concat_zeros]
        jax.block_until_ready(dev_zeros)
        t0 = time.perf_counter()
        outs = sharded(*dev_in, *dev_zeros)
        jax.block_until_ready(outs)
        return outs, time.perf_counter() - t0

    def results(outs):
        res = []
        for c in range(n_cores):
            d = {}
            for i, nm in enumerate(out_names):
                per = np.asarray(outs[i])
                rows = per.shape[0] // n_cores
                d[nm] = per[c * rows:(c + 1) * rows]
            res.append(d)
        return res

    return run, results
